# revision 1
# baseline (speedup 1.0000x reference)
# kernel.py — DyResConv_Inf (moe_routing) on 8 TRN2 NeuronCores.
#
# Reference computation:
#   r = routing(x)                      # (3, 768) sigmoid gates from global pools of x
#   w = sum_e r[e,o] * convs[e,o,:,:,:] # fused 3x3 conv weight synthesis
#   y = conv2d(x, w, stride 1, pad 1)   # (1, 768, 120, 120)
#
# One SPMD program on 8 cores; all per-core variation is input *data*
# prepared on the host. Core c (q = c//2 row-quarter, h = c%2 channel-half)
# computes y for out-channels [384h, 384h+384) x rows [30q, 30q+30), so
# every conv matmul uses the full M=128 PE width (3 o-blocks of 128).
#
# Routing head (latency-critical, overlapped with weight prefetch):
#   - each core pools rows [15c, 15c+15): ONE matmul per row against a
#     per-core mask matrix (input data) that fuses 24/40-column grouping
#     AND the row->pool-block "slot" masking; PSUM accumulates over rows
#   - the [16, 768] slot sums are AllGathered (24KB bf16 per core)
#   - pool-block reconstruction, the bicubic 3->5 upsample, and the
#     transposes back to channel-major all run as PE matmuls/transposes
#     on the (core,slot)-major gather — the DVE does almost nothing
#   - pool-mean divisions are folded into w_pw1 columns on the host
#
# Weight path: gates are applied on DVE/ACT in natural o-major layout
# (tensor_scalar with the per-partition r column), then true PE transposes
# (fp32 transpose mode, 2 cyc/row) produce wT[cin-tap, o] tiles.
#
# Main conv runs in float32r — full PE rate (1 cyc/row) at N>=256, vs 4
# cyc/row for plain fp32 — with fp32 PSUM accumulation: per o-block,
# 2 passes x 4 PSUM banks x 54 k-tiles (tap-major: the host pre-reorders
# convs to [e, o, (tap, cin)]), rhs = 2D-AP row windows into the
# host-padded x quarter slice. Measured on hardware: abs max err 1.1e-3
# on an 8.7-scale output (1.3e-4 scale-relative).
#
import os

import numpy as np

os.environ.setdefault("MYCRO_LOCAL_CACHE", "1")

N_CORES = 8
C = 768          # in = out channels
H = W = 120
S = 48           # squeeze channels
E = 3            # experts
KK = 3           # kernel size
NB = 6           # channel blocks of 128 (768/128)
OBPC = 3         # o-blocks per core
QR = 30          # rows per quarter
PR = 15          # pool rows per core
NK = 54          # k-tiles: 9 taps x 6 cin-blocks
NDT = 6          # cv DMA tiles per (e, lob): 9 chunks each
DTC = 9          # chunks per cv DMA tile
F32 = np.float32

SYNTH_MODE = os.environ.get("KBENCH_SYNTH", "dve")  # dve | pe | bf16
ILV = bool(int(os.environ.get("KBENCH_ILV", "0")))
SYNTH_BF16 = SYNTH_MODE == "bf16"

# conv output row chunks per pass: (local_row_start, n_rows)
PASS_CHUNKS = [
    [(0, 4), (4, 4), (8, 4), (12, 3)],
    [(15, 4), (19, 4), (23, 4), (27, 3)],
]


def _bicubic_mat(in_size, out_size):
    """PyTorch-style bicubic (a=-0.75), align_corners=False, border-replicate."""
    a = -0.75

    def k(x):
        x = abs(x)
        if x <= 1.0:
            return (a + 2) * x**3 - (a + 3) * x**2 + 1.0
        if x < 2.0:
            return a * x**3 - 5 * a * x**2 + 8 * a * x - 4 * a
        return 0.0

    M = np.zeros((out_size, in_size), dtype=F32)
    scale = in_size / out_size
    for j in range(out_size):
        src = (j + 0.5) * scale - 0.5
        i0 = int(np.floor(src))
        t = src - i0
        for m in range(-1, 3):
            idx = min(max(i0 + m, 0), in_size - 1)
            M[j, idx] += k(m - t)
    return M


def _slot_terms(blocksize, nblocks):
    """For each pool block: list of (core, slot) contributing partial sums."""
    terms = [[] for _ in range(nblocks)]
    for c in range(N_CORES):
        base = (PR * c) // blocksize
        for s in range(2):
            b = base + s
            if b >= nblocks:
                continue
            lo, hi = max(PR * c, blocksize * b), min(PR * c + PR, blocksize * (b + 1))
            if lo < hi:
                terms[b].append((c, s))
    return terms


_prog_cache = {}


def _get_program(iters=None):
    """Build (once per iters) the SPMD Bass/Tile program. Returns nc."""
    global _prog_cache
    if iters is None:
        iters = int(os.environ.get("KBENCH_ITERS", "1"))
    if iters in _prog_cache:
        return _prog_cache[iters]

    from contextlib import ExitStack

    import concourse.bass as bass
    import concourse.tile as tile
    from concourse import bacc, mybir

    f32 = mybir.dt.float32
    f32r = mybir.dt.float32r
    bf16 = mybir.dt.bfloat16
    sdt = bf16 if SYNTH_BF16 else f32
    AX = mybir.AxisListType
    ALU = mybir.AluOpType
    ACT = mybir.ActivationFunctionType

    nc = bacc.Bacc(
        "TRN2",
        target_bir_lowering=False,
        debug=False,
        enable_asserts=False,
        num_devices=N_CORES,
    )

    # ---- I/O tensors (per-core contents differ; shapes identical) ----
    xq_d = nc.dram_tensor("xq", [C, 32, 122], f32r, kind="ExternalInput").ap()
    xpt_d = nc.dram_tensor("xpt", [PR, W, C], bf16, kind="ExternalInput").ap()
    cv_d = nc.dram_tensor("cvs", [E, 384, NK * 128], sdt, kind="ExternalInput").ap()
    cf32_d = nc.dram_tensor("cf32", [128, 18 * S + 25 + 128], f32,
                            kind="ExternalInput").ap()
    c48_d = nc.dram_tensor("c48", [S, 9 * 128 + 9 + 9 + 25], f32,
                           kind="ExternalInput").ap()
    sel_d = nc.dram_tensor("selmm", [128, 64], bf16, kind="ExternalInput").ap()
    mask_d = nc.dram_tensor("maskmm", [W, PR, 16], bf16,
                            kind="ExternalInput").ap()
    y_d = nc.dram_tensor("y_out", [384, QR, W], f32, kind="ExternalOutput").ap()

    with tile.TileContext(nc) as tc, ExitStack() as ctx:
        consts = ctx.enter_context(tc.tile_pool(name="consts", bufs=1))
        xqp = ctx.enter_context(tc.tile_pool(name="xqp", bufs=1))
        small = ctx.enter_context(tc.tile_pool(name="small", bufs=1))
        dram = ctx.enter_context(tc.tile_pool(name="dram", bufs=1, space="DRAM"))
        psy = ctx.enter_context(tc.tile_pool(name="psy", bufs=4, space="PSUM"))
        pss = ctx.enter_context(tc.tile_pool(name="pss", bufs=2, space="PSUM"))
        cvp = ctx.enter_context(tc.tile_pool(name="cvp", bufs=2))
        # per-iteration body (iters>1 is used only for wall-clock benchmarking)
        for _it in range(iters):
          rctx = ExitStack()
          xpp = rctx.enter_context(tc.tile_pool(name=f"xpp{_it}", bufs=1))

          # ---- pool-critical transfers first, then merged constants ----
          maskmm = xpp.tile([W, PR * 16], bf16, tag="maskmm")
          nc.sync.dma_start(maskmm[:], mask_d)

          # ---- x pool rows ----
          xpt_sb = xpp.tile([W, PR * C], bf16, tag="xpt")
          xptv = xpt_sb[:].rearrange("w (r c) -> w r c", r=PR)
          nc.sync.dma_start(xptv[:, 0:8, :],
                            xpt_d[0:8, :, :].rearrange("r w c -> w r c"))
          nc.sync.dma_start(xptv[:, 8:PR, :],
                            xpt_d[8:PR, :, :].rearrange("r w c -> w r c"))

          cf32 = consts.tile([128, 18 * S + 25 + 128], f32, tag="cf32")
          nc.sync.dma_start(cf32[:], cf32_d)
          w1t = cf32[:, 0:18 * S]
          ones25 = cf32[:, 18 * S:18 * S + 25]
          ident = cf32[:, 18 * S + 25:]
          c48 = consts.tile([S, 9 * 128 + 9 + 9 + 25], f32, tag="c48")
          nc.sync.dma_start(c48[:], c48_d)
          w2t = c48[:, 0:9 * 128]
          wdw1 = c48[:, 9 * 128:9 * 128 + 9]
          wdw2 = c48[:, 9 * 128 + 9:9 * 128 + 18]
          tkmm = c48[0:9, 9 * 128 + 18:9 * 128 + 43]
          selmm = consts.tile([128, 64], bf16, tag="selmm")
          nc.sync.dma_start(selmm[:], sel_d)
          actwarm = small.tile([1, 1], f32, tag="actwarm")
          nc.scalar.activation(actwarm[:], ident[0:1, 0:1], ACT.Sigmoid)
          nc.scalar.activation(actwarm[:], ident[0:1, 0:1], ACT.Relu)
          nc.scalar.activation(actwarm[:], ident[0:1, 0:1], ACT.Copy,
                               scale=ident[0:1, 0:1])


          # PE warmup: ramp the clock gate before the latency-critical
          # pool matmuls (junk results, never read)
          junkp = pss.tile([128, 128], f32, tag="pw", bufs=2, name=f"jk{_it}")
          for _w in range(2):
              nc.tensor.matmul(junkp[:], ident, ident,
                               start=True, stop=True, skip_group_check=True)

          # ---- stage-1 pools as PE matmuls: payload[pat, c] ----
          # lhsT = per-core mask matrix fusing column-grouping and row-slot
          # masks; rhs = transposed pool rows; PSUM accumulates over rows.
          ppay = [pss.tile([16, 384], f32, tag="prt", bufs=2,
                           name=f"ppay{h2}_{_it}") for h2 in range(2)]
          for row in range(PR):
              for h2 in range(2):
                  nc.tensor.matmul(
                      ppay[h2][:],
                      maskmm[:, row * 16:(row + 1) * 16],
                      xptv[:, row, h2 * 384:(h2 + 1) * 384],
                      start=(row == 0), stop=(row == PR - 1),
                  )
          payload = xpp.tile([16, C], bf16, tag="payload")
          for h2 in range(2):
              nc.scalar.activation(payload[:, h2 * 384:(h2 + 1) * 384],
                                   ppay[h2][:], ACT.Copy)

          # ---- AllGather the slot sums (49KB in, 393KB out) ----
          cc_in = dram.tile([16, C], bf16, tag="ccin")
          cc_out = dram.tile([N_CORES, 16, C], bf16, tag="ccout",
                             addr_space="Shared")
          cc_dma = nc.sync.dma_start(cc_in[:], payload[:])
          nc.gpsimd.collective_compute(
              "AllGather",
              ALU.bypass,
              replica_groups=[list(range(N_CORES))],
              ins=[cc_in[:].opt()],
              outs=[cc_out[:].opt()],
          )
          xq_sb = []
          xqb_dmas = []
          for p in range(NB):
              t = xqp.tile([128, 32 * 122], f32r, tag=f"xq{p}")
              da_ = nc.sync.dma_start(
                  t[:, 0:18 * 122], xq_d[p * 128:(p + 1) * 128, 0:18, :]
              )
              tile.add_dep_helper(da_.ins, cc_dma.ins, sync=True,
                                  reason="xq passA after pool payload")
              xqb_dmas.append(nc.sync.dma_start(
                  t[:, 18 * 122:], xq_d[p * 128:(p + 1) * 128, 18:32, :]
              ))
              xq_sb.append(t)

          # prefetch the expert-weight stream (no deps; gpsimd queue)
          cv_tiles = {}
          for lob in range(OBPC):
              for dt_i in range(NDT):
                  tl = []
                  for e in range(E):
                      t = cvp.tile([128, DTC * 128], sdt, tag=f"cv{e}",
                                   name=f"cv{e}_{lob}_{dt_i}_{_it}")
                      di = nc.gpsimd.dma_start(
                          t[:],
                          cv_d[e, lob * 128:(lob + 1) * 128,
                               dt_i * DTC * 128:(dt_i + 1) * DTC * 128],
                      )
                      if lob == 0 and dt_i == 0 and e == 0:
                          # keep the 32MB expert stream from congesting the
                          # DMA movers before the pool payload is out
                          tile.add_dep_helper(di.ins, cc_dma.ins, sync=True,
                                              reason="cv after payload")
                      tl.append(t)
                  cv_tiles[(lob, dt_i)] = tl

          # one trivial DMA: gathered slot sums, (core,pat) on partitions
          g2pm = xpp.tile([128, C], bf16, tag="g2pm")
          g2pm_dma = nc.sync.dma_start(
              g2pm[:], cc_out[:].rearrange("core pat c -> (core pat) c"))
          for d_ in xqb_dmas:
              # keep the movers free for the tiny routing-critical readback
              tile.add_dep_helper(d_.ins, g2pm_dma.ins, sync=True,
                                  reason="xq passB after gather readback")

          # pool block sums via SEL matmuls (a5+a1 | a3 separately, both
          # based at partition 0 so downstream matmuls can consume them)
          stg = xpp.tile([32, C], f32, tag="stg")
          stg3 = xpp.tile([9, C], f32, tag="stg3")
          stgu = xpp.tile([25, C], f32, tag="stgu")
          for h2 in range(2):
              psel = pss.tile([32, 384], f32, tag="prt", bufs=2,
                              name=f"psel{h2}_{_it}")
              nc.tensor.matmul(psel[:], selmm[:][:, 0:32],
                               g2pm[:, h2 * 384:(h2 + 1) * 384],
                               start=True, stop=True)
              nc.scalar.activation(stg[:, h2 * 384:(h2 + 1) * 384], psel[:],
                                   ACT.Copy)
              psel3 = pss.tile([9, 384], f32, tag="prt", bufs=2,
                               name=f"psel3{h2}_{_it}")
              nc.tensor.matmul(psel3[:], selmm[:][:, 32:41],
                               g2pm[:, h2 * 384:(h2 + 1) * 384],
                               start=True, stop=True)
              nc.scalar.activation(stg3[:, h2 * 384:(h2 + 1) * 384], psel3[:],
                                   ACT.Copy)
          # bicubic 3->5 as a matmul in transposed space: a3uT = Tk.T @ a3T
          for h2 in range(2):
              pbic = pss.tile([25, 384], f32, tag="prt", bufs=2,
                              name=f"pbic{h2}_{_it}")
              nc.tensor.matmul(pbic[:], tkmm,
                               stg3[:, h2 * 384:(h2 + 1) * 384],
                               start=True, stop=True)
              nc.scalar.activation(stgu[:, h2 * 384:(h2 + 1) * 384], pbic[:],
                                   ACT.Copy)

          # transpose per pblock back to channel-major
          att = small.tile([128, 18 * 25], f32, tag="att")
          att5 = xpp.tile([128, NB * 32], f32, tag="att5")
          for p in range(NB):
              pt1 = pss.tile([128, 32], f32, tag="pw", bufs=2,
                             name=f"pt1_{p}_{_it}")
              nc.tensor.transpose(pt1[:], stg[:, p * 128:(p + 1) * 128],
                                  ident[0:32, 0:32])
              nc.scalar.activation(att5[:, p * 32:(p + 1) * 32], pt1[:], ACT.Copy)
              pt2 = pss.tile([128, 25], f32, tag="pw", bufs=2,
                             name=f"pt2_{p}_{_it}")
              nc.tensor.transpose(pt2[:], stgu[:, p * 128:(p + 1) * 128],
                                  ident[0:25, 0:25])
              nc.scalar.activation(att[:, (6 + p) * 25:(7 + p) * 25], pt2[:],
                                   ACT.Copy)
              # a1e block: ones * total (raw sums; scaling folded into w_pw1)
              nc.vector.tensor_scalar_mul(
                  att[:, p * 25:(p + 1) * 25], ones25,
                  att5[:, p * 32 + 25:p * 32 + 26])

          # ---- routing net ----
          ph = pss.tile([S, 25], f32, tag="prt", bufs=2)
          for j in range(18):
              rhs = (att5[:, (j - 12) * 32:(j - 12) * 32 + 25] if j >= 12
                     else att[:, j * 25:(j + 1) * 25])
              nc.tensor.matmul(
                  ph[:],
                  w1t[:, j * S:(j + 1) * S],
                  rhs,
                  start=(j == 0), stop=(j == 17),
              )
          hdd1 = xpp.tile([S, 25], f32, tag="hdd1")
          nc.scalar.activation(hdd1[:], ph[:], ACT.Relu)

          hdd2 = xpp.tile([S, 9], f32, tag="hdd2")
          h1v = hdd1[:].rearrange("s (p q) -> s p q", p=5)
          for uv in range(9):
              u, v = uv // 3, uv % 3
              if uv == 0:
                  nc.vector.tensor_scalar_mul(
                      hdd2[:].rearrange("s (p q) -> s p q", p=3),
                      h1v[:, u:u + 3, v:v + 3], wdw1[:, 0:1]
                  )
              else:
                  t9 = xpp.tile([S, 9], f32, tag="t9", name=f"t9_{uv}")
                  nc.vector.tensor_scalar_mul(
                      t9[:].rearrange("s (p q) -> s p q", p=3),
                      h1v[:, u:u + 3, v:v + 3], wdw1[:, uv:uv + 1]
                  )
                  nc.vector.tensor_tensor(
                      out=hdd2[:], in0=hdd2[:], in1=t9[:], op=ALU.add
                  )
          nc.scalar.activation(hdd2[:], hdd2[:], ACT.Relu)

          t9b = xpp.tile([S, 9], f32, tag="t9b")
          nc.vector.tensor_tensor(out=t9b[:], in0=hdd2[:], in1=wdw2[:], op=ALU.mult)
          hdd3 = xpp.tile([S, 1], f32, tag="hdd3")
          nc.vector.tensor_reduce(hdd3[:], t9b[:], axis=AX.X, op=ALU.add)
          nc.scalar.activation(hdd3[:], hdd3[:], ACT.Relu)

          pr = pss.tile([128, 9], f32, tag="prt", bufs=2)
          for m in range(9):
              nc.tensor.matmul(
                  pr[:, m:m + 1],
                  w2t[:, m * 128:(m + 1) * 128],
                  hdd3[:],
                  start=True, stop=True, skip_group_check=True,
              )
          r_sb = small.tile([128, 9], f32, tag="r_sb")
          nc.scalar.activation(r_sb[:], pr[:], ACT.Sigmoid)

          if SYNTH_MODE != "dve":
              # ---- diag gate matrices (dtype follows synthesis dtype) ----
              dall = small.tile([128, 9 * 128], sdt, tag="dall")
              for col in range(9):
                  nc.vector.tensor_scalar_mul(
                      dall[:, col * 128:(col + 1) * 128], ident[:],
                      r_sb[:, col:col + 1]
                  )

          rctx.close()  # free routing-phase SBUF before the conv phase
          ictx = ExitStack()
          wtp = ictx.enter_context(tc.tile_pool(name=f"wtp{_it}", bufs=1))
          wsp = ictx.enter_context(tc.tile_pool(name=f"wsp{_it}", bufs=2))

          # ---- x quarter (f32r view), split DMA so pass A can start early ----

          # ---- per o-block: weight synthesis (PE, transposed) + main conv ----
          for lob in range(OBPC):
              wt = wtp.tile([128, NK * 128], f32r, tag="wt")
              for dt_i in range(NDT):
                  cvt = cv_tiles[(lob, dt_i)]
                  if SYNTH_MODE == "dve":
                      # gate on DVE+ACT in o-major (per-partition scalar =
                      # r col), then true PE transposes into wT
                      wsb = wsp.tile([128, DTC * 128], f32, tag="wsb",
                                 bufs=int(os.environ.get("KBENCH_WSB", "4")),
                                     name=f"wsb{lob}_{dt_i}")
                      nc.vector.tensor_scalar_mul(
                          wsb[:], cvt[0][:], r_sb[:, lob:lob + 1])
                      tmp1 = wsp.tile([128, DTC * 128], f32, tag="wtmp1",
                                      bufs=2, name=f"wtmp1_{lob}_{dt_i}")
                      nc.scalar.activation(tmp1[:], cvt[1][:], ACT.Copy,
                                           scale=r_sb[:, 3 + lob:4 + lob])
                      tmp2 = wsp.tile([128, DTC * 128], f32, tag="wtmp2",
                                      bufs=2, name=f"wtmp2_{lob}_{dt_i}")
                      nc.vector.tensor_scalar_mul(
                          tmp2[:], cvt[2][:], r_sb[:, 6 + lob:7 + lob])
                      nc.vector.tensor_tensor(
                          out=wsb[:], in0=wsb[:], in1=tmp1[:], op=ALU.add)
                      nc.vector.tensor_tensor(
                          out=wsb[:], in0=wsb[:], in1=tmp2[:], op=ALU.add)
                      for ci in range(DTC):
                          k = dt_i * DTC + ci
                          pw = pss.tile([128, 128], f32, tag="pw", bufs=2,
                                        name=f"pw{lob}_{k}")
                          nc.tensor.transpose(
                              pw[:], wsb[:, ci * 128:(ci + 1) * 128], ident[:])
                          nc.scalar.activation(wt[:, k * 128:(k + 1) * 128],
                                               pw[:], ACT.Copy)
                      # interleave conv pass A with synthesis so the PE never
                      # waits a whole synth phase before starting the conv
                      if not ILV:
                          continue
                      if dt_i == 0:
                          passA = [
                              psy.tile([128, nr * W], f32, tag="py",
                                       name=f"pyA{lob}_{r0}")
                              for (r0, nr) in PASS_CHUNKS[0]
                          ]
                      for k in range(dt_i * DTC, (dt_i + 1) * DTC):
                          uv, p = k // NB, k % NB
                          u, v = uv // 3, uv % 3
                          xv = xq_sb[p][:].rearrange("c (r w) -> c r w", r=32)
                          for ci2, (r0, nr) in enumerate(PASS_CHUNKS[0]):
                              nc.tensor.matmul(
                                  passA[ci2][:],
                                  wt[:, k * 128:(k + 1) * 128],
                                  xv[:, r0 + u:r0 + u + nr, v:v + W],
                                  start=(k == 0), stop=(k == NK - 1),
                              )
                  else:
                      for ci in range(DTC):
                          k = dt_i * DTC + ci
                          pw = pss.tile([128, 128], f32, tag="pw", bufs=2,
                                        name=f"pw{lob}_{k}")
                          for e in range(E):
                              dsl = dall[:, (e * 3 + lob) * 128:(e * 3 + lob + 1) * 128]
                              nc.tensor.matmul(
                                  pw[:], cvt[e][:, ci * 128:(ci + 1) * 128], dsl,
                                  start=(e == 0), stop=(e == E - 1),
                              )
                          if k % 2 == 0:
                              nc.vector.tensor_copy(wt[:, k * 128:(k + 1) * 128], pw[:])
                          else:
                              nc.scalar.activation(wt[:, k * 128:(k + 1) * 128], pw[:],
                                                   ACT.Copy)

              # conv: pass A was interleaved with synthesis in dve mode
              passes = ([(PASS_CHUNKS[0], passA, "drain_only"),
                         (PASS_CHUNKS[1], None, "full")]
                        if (SYNTH_MODE == "dve" and ILV) else
                        [(PASS_CHUNKS[0], None, "full"),
                         (PASS_CHUNKS[1], None, "full")])
              for chunks, pys, pmode in passes:
                  if pys is None:
                      pys = [
                          psy.tile([128, nr * W], f32, tag="py",
                                   name=f"py{lob}_{r0}")
                          for (r0, nr) in chunks
                      ]
                      for k in range(NK):
                          uv, p = k // NB, k % NB
                          u, v = uv // 3, uv % 3
                          xv = xq_sb[p][:].rearrange("c (r w) -> c r w", r=32)
                          for ci, (r0, nr) in enumerate(chunks):
                              nc.tensor.matmul(
                                  pys[ci][:],
                                  wt[:, k * 128:(k + 1) * 128],
                                  xv[:, r0 + u:r0 + u + nr, v:v + W],
                                  start=(k == 0), stop=(k == NK - 1),
                              )
                  for ci, (r0, nr) in enumerate(chunks):
                      ysb = small.tile(
                          [128, nr * W], f32, tag="ysb", bufs=3,
                          name=f"ysb{lob}_{r0}",
                      )
                      nc.scalar.activation(ysb[:], pys[ci][:], ACT.Copy)
                      nc.sync.dma_start(
                          y_d[lob * 128:(lob + 1) * 128, r0:r0 + nr, :], ysb[:]
                      )
          ictx.close()

    nc.finalize()
    _prog_cache[iters] = nc
    return nc


def prepare_in_maps(x, convs, w_pw1, w_dw1, w_dw2, w_pw2):
    """Host-side slicing/layout prep. Returns list of 8 per-core input dicts."""
    import ml_dtypes

    x = np.asarray(x, dtype=F32)
    convs = np.asarray(convs, dtype=F32)
    w_pw1 = np.asarray(w_pw1, dtype=F32)
    w_dw1 = np.asarray(w_dw1, dtype=F32)
    w_dw2 = np.asarray(w_dw2, dtype=F32)
    w_pw2 = np.asarray(w_pw2, dtype=F32)

    x0 = x[0]  # (768, 120, 120)
    xpad = np.zeros((C, H + 2, W + 2), dtype=F32)
    xpad[:, 1:H + 1, 1:W + 1] = x0

    # convs -> [e, o, (u, v, cin)]
    cvr = np.ascontiguousarray(
        convs.transpose(0, 1, 3, 4, 2).reshape(E, C, KK * KK * C)
    )
    if SYNTH_BF16:
        cvr = cvr.astype(ml_dtypes.bfloat16)

    # w_pw1 prescaled by pool-mean factors, transposed, pblock-major
    colscale = np.concatenate([
        np.full(C, 1.0 / (H * W), dtype=F32),
        np.full(C, 1.0 / 1600.0, dtype=F32),
        np.full(C, 1.0 / 576.0, dtype=F32),
    ])
    w1s = (w_pw1 * colscale[None, :]).astype(F32)          # (48, 2304)
    w1t = np.ascontiguousarray(
        w1s.T.reshape(18, 128, S).transpose(1, 0, 2).reshape(128, 18 * S)
    )

    M35 = _bicubic_mat(3, 5)                                # (5, 3)
    tkmm = np.ascontiguousarray(
        np.einsum("pi,qj->ijpq", M35, M35).reshape(9, 25)).astype(F32)
    sel = np.zeros((128, 64), dtype=F32)
    for b, terms in enumerate(_slot_terms(24, 5)):
        for (c_, s_) in terms:
            for z in range(5):
                sel[c_ * 16 + s_ * 8 + z, b * 5 + z] = 1.0
                sel[c_ * 16 + s_ * 8 + z, 25] = 1.0      # a1 = sum of all a5
    for b, terms in enumerate(_slot_terms(40, 3)):
        for (c_, s_) in terms:
            for z in range(3):
                sel[c_ * 16 + s_ * 8 + 5 + z, 32 + b * 3 + z] = 1.0
    import ml_dtypes as _mld
    sel = sel.astype(_mld.bfloat16)

    ones25 = np.ones((128, 25), dtype=F32)
    ident = np.eye(128, dtype=F32)
    wdw1 = np.ascontiguousarray(w_dw1.reshape(S, 9))
    wdw2 = np.ascontiguousarray(w_dw2.reshape(S, 9))

    in_maps = []
    for c in range(N_CORES):
        q, h = c // 2, c % 2
        xq = np.ascontiguousarray(xpad[:, 30 * q:30 * q + 32, :])
        xpt = np.ascontiguousarray(
            x0[:, PR * c:PR * (c + 1), :].transpose(1, 2, 0)
        ).astype(ml_dtypes.bfloat16)                       # (15, 120, 768)
        maskmm = np.zeros((PR, W, 16), dtype=F32)
        for r_ in range(PR):
            grow = PR * c + r_
            for col in range(W):
                pc5, pc3 = col // 24, col // 40
                for s_ in range(2):
                    if grow // 24 == (PR * c) // 24 + s_:
                        maskmm[r_, col, s_ * 8 + pc5] = 1.0
                    if grow // 40 == (PR * c) // 40 + s_:
                        maskmm[r_, col, s_ * 8 + 5 + pc3] = 1.0
        maskmm = np.ascontiguousarray(
            maskmm.transpose(1, 0, 2)).astype(ml_dtypes.bfloat16)
        cvs = np.ascontiguousarray(cvr[:, 384 * h:384 * (h + 1), :])
        w2t = np.empty((S, 9 * 128), dtype=F32)
        for e in range(E):
            for lob in range(OBPC):
                rows = slice(e * C + (3 * h + lob) * 128,
                             e * C + (3 * h + lob) * 128 + 128)
                w2t[:, (e * 3 + lob) * 128:(e * 3 + lob + 1) * 128] = w_pw2[rows, :].T
        cf32 = np.concatenate([w1t, ones25, ident], axis=1)
        c48 = np.concatenate(
            [w2t, wdw1, wdw2,
             np.concatenate([tkmm, np.zeros((S - 9, 25), dtype=F32)], axis=0)],
            axis=1)
        in_maps.append({
            "xq": xq, "xpt": xpt, "cvs": cvs, "cf32": cf32, "c48": c48,
            "selmm": sel, "maskmm": maskmm,
        })
    return in_maps


def reassemble(outs):
    """outs: list of 8 dicts with 'y_out' (384, 30, 120) -> (1, 768, 120, 120)."""
    y = np.empty((1, C, H, W), dtype=F32)
    for c in range(N_CORES):
        q, h = c // 2, c % 2
        y[0, 384 * h:384 * (h + 1), 30 * q:30 * (q + 1), :] = outs[c]["y_out"]
    return y


last_results = None  # BassKernelResults from the most recent run (for test.py)


def kernel(x, convs, w_pw1, w_dw1, w_dw2, w_pw2):
    global last_results
    from concourse import bass_utils

    nc = _get_program()
    in_maps = prepare_in_maps(x, convs, w_pw1, w_dw1, w_dw2, w_pw2)
    trace = bool(int(os.environ.get("KBENCH_TRACE", "0")))
    res = bass_utils.run_bass_kernel_spmd(
        nc, in_maps, core_ids=list(range(N_CORES)), trace=trace,
    )
    last_results = res
    return reassemble(res.results)

